# revision 2
# baseline (speedup 1.0000x reference)
"""Trainium2 Bass kernel for nn_EncoderSimilarity (block-cosine similarity).

sims[a,b] = sum over block-granularities {128, 256} of
            sum_t max_v ( l2norm(img_block_v) . l2norm(cap_block_t) )

Sharding: img rows (axis a) split 8 ways across cores, cap replicated;
each core computes its [256, 2048] slice of sims.

Device algorithm per core:
  - Block-l2-normalize img slice and cap at granularities 128/256 (the
    reference's global cap l2norm cancels inside the block norm; error ~1e-9).
  - Cast normalized operands to bf16, transpose to [c, b] layout via DMA
    xbar transpose (DRAM round-trip) so the contraction dim is on partitions.
  - Logits via bf16 matmuls; max-over-v uses a relu-diff decomposition
      max(L0, L1) = L1 + relu(L0 - L1),
    where L0-L1 comes directly from a matmul with differenced img weights,
    so ScalarE (relu) shares the PSUM drain work with VectorE (add/max).
  - t-sums accumulate in fp32 via a strided reduce over staged bf16 maxes.
"""
import sys

if "/opt/trn_rl_repo" not in sys.path:
    sys.path.insert(0, "/opt/trn_rl_repo")

from contextlib import ExitStack

import numpy as np

N_CORES = 8
A, B, C = 2048, 2048, 1024
A_PER = A // N_CORES          # 256 img rows per core
NQ = 4                        # b processed in quarters of 512
BQ = B // NQ                  # 512


def _build_kernel():
    import concourse.bass as bass
    import concourse.tile as tile
    from concourse import mybir

    F32 = mybir.dt.float32
    BF16 = mybir.dt.bfloat16
    Alu = mybir.AluOpType
    Act = mybir.ActivationFunctionType
    Ax = mybir.AxisListType

    nc = bass.Bass(
        trn_type="TRN2",
        target_bir_lowering=False,
        debug=False,
        num_devices=N_CORES,
    )
    img_d = nc.dram_tensor("img", [A_PER, C], F32, kind="ExternalInput").ap()
    cap_d = nc.dram_tensor("cap", [B, C], F32, kind="ExternalInput").ap()
    out_d = nc.dram_tensor("sims", [A_PER, B], F32, kind="ExternalOutput").ap()

    with tile.TileContext(nc) as tc, ExitStack() as ctx:
        _body(ctx, tc, out_d, img_d, cap_d, F32, BF16, Alu, Act, Ax)
    return nc


def _body(ctx, tc, out_d, img_d, cap_d, F32, BF16, Alu, Act, Ax):
    import concourse.bass as bass
    nc = tc.nc

    dram = ctx.enter_context(tc.tile_pool(name="dram", bufs=1, space="DRAM"))
    persist = ctx.enter_context(tc.tile_pool(name="persist", bufs=1))
    norm = ctx.enter_context(tc.tile_pool(name="norm", bufs=3))
    small = ctx.enter_context(tc.tile_pool(name="small", bufs=3))
    stage = ctx.enter_context(tc.tile_pool(name="stage", bufs=2))
    drain = ctx.enter_context(tc.tile_pool(name="drain", bufs=3))
    psum = ctx.enter_context(tc.tile_pool(name="psum", bufs=2, space="PSUM"))

    # ---------------- normalization helper (natural [n, c] layout) -------------
    def normalize_tile(x_f32, n128_out, n256_out):
        """x_f32: [128, 1024] fp32 -> block-l2-normalized bf16 tiles (128/256)."""
        sq = norm.tile([128, C], F32, tag="sq")
        nc.scalar.activation(sq[:], x_f32[:], Act.Square)
        s128 = small.tile([128, 8], F32, tag="s128")
        nc.vector.reduce_sum(
            s128[:], sq.rearrange("p (j c) -> p j c", c=128), axis=Ax.X
        )
        s256 = small.tile([128, 4], F32, tag="s256")
        nc.vector.tensor_tensor(
            s256[:],
            s128.rearrange("p (k two) -> p k two", two=2)[:, :, 0],
            s128.rearrange("p (k two) -> p k two", two=2)[:, :, 1],
            op=Alu.add,
        )
        rt128 = small.tile([128, 8], F32, tag="rt128")
        nc.scalar.activation(rt128[:], s128[:], Act.Sqrt)
        inv128 = small.tile([128, 8], F32, tag="inv128")
        nc.vector.reciprocal(inv128[:], rt128[:])
        rt256 = small.tile([128, 4], F32, tag="rt256")
        nc.scalar.activation(rt256[:], s256[:], Act.Sqrt)
        inv256 = small.tile([128, 4], F32, tag="inv256")
        nc.vector.reciprocal(inv256[:], rt256[:])
        for j in range(8):
            nc.vector.tensor_scalar_mul(
                n128_out[:, j * 128:(j + 1) * 128],
                x_f32[:, j * 128:(j + 1) * 128],
                inv128[:, j:j + 1],
            )
        for k in range(4):
            nc.vector.tensor_scalar_mul(
                n256_out[:, k * 256:(k + 1) * 256],
                x_f32[:, k * 256:(k + 1) * 256],
                inv256[:, k:k + 1],
            )

    # ---------------- img prep -> transposed bf16 weight tiles -----------------
    # normalized img in natural layout
    img_n128 = persist.tile([128, 2, C], BF16, tag="img_n128")   # [a-tile][a, c]
    img_n256 = persist.tile([128, 2, C], BF16, tag="img_n256")
    img_dn128 = persist.tile([128, 2, 512], BF16, tag="img_dn128")  # 4 pair-diffs
    img_dn256 = persist.tile([128, 2, 512], BF16, tag="img_dn256")  # 2 pair-diffs x 256
    for at in range(2):
        x = norm.tile([128, C], F32, tag="img_in")
        nc.sync.dma_start(x[:], img_d[at * 128:(at + 1) * 128, :])
        normalize_tile(x, img_n128[:, at, :], img_n256[:, at, :])
        # pair diffs on normalized bf16 data (even - odd blocks)
        nc.vector.tensor_tensor(
            img_dn128.rearrange("p t (i c) -> p t i c", c=128)[:, at],
            img_n128.rearrange("p t (v c) -> p t v c", c=128)[:, at, 0::2, :],
            img_n128.rearrange("p t (v c) -> p t v c", c=128)[:, at, 1::2, :],
            op=Alu.subtract,
        )
        nc.vector.tensor_tensor(
            img_dn256.rearrange("p t (i c) -> p t i c", c=256)[:, at],
            img_n256.rearrange("p t (v c) -> p t v c", c=256)[:, at, 0::2, :],
            img_n256.rearrange("p t (v c) -> p t v c", c=256)[:, at, 1::2, :],
            op=Alu.subtract,
        )

    # stage img to DRAM and transpose back to [c, a] layout
    scr_i128 = dram.tile([A_PER, C], BF16, tag="scr_i128")
    scr_i256 = dram.tile([A_PER, C], BF16, tag="scr_i256")
    scr_d128 = dram.tile([A_PER, 512], BF16, tag="scr_d128")
    scr_d256 = dram.tile([A_PER, 512], BF16, tag="scr_d256")
    for at in range(2):
        sl = slice(at * 128, (at + 1) * 128)
        nc.sync.dma_start(scr_i128[sl, :], img_n128[:, at, :])
        nc.sync.dma_start(scr_i256[sl, :], img_n256[:, at, :])
        nc.sync.dma_start(scr_d128[sl, :], img_dn128[:, at, :])
        nc.sync.dma_start(scr_d256[sl, :], img_dn256[:, at, :])

    # weight tiles, [c, a] layout: index i = pair 0..3
    wL128 = persist.tile([128, 4, A_PER], BF16, tag="wL128")  # odd chunk 2i+1
    wD128 = persist.tile([128, 4, A_PER], BF16, tag="wD128")
    wL256 = persist.tile([128, 4, A_PER], BF16, tag="wL256")  # [2i+h]: odd v'=2i+1, half h
    wD256 = persist.tile([128, 4, A_PER], BF16, tag="wD256")
    for i in range(4):
        j = 2 * i + 1  # odd 128-chunk
        nc.sync.dma_start_transpose(wL128[:, i, :], scr_i128[:, j * 128:(j + 1) * 128])
        nc.sync.dma_start_transpose(wD128[:, i, :], scr_d128[:, i * 128:(i + 1) * 128])
    for i in range(2):       # pair of 256-blocks: odd v' = 2i+1
        for h in range(2):   # 128-half of the 256-block
            j = (2 * i + 1) * 2 + h
            nc.sync.dma_start_transpose(
                wL256[:, 2 * i + h, :], scr_i256[:, j * 128:(j + 1) * 128]
            )
            nc.sync.dma_start_transpose(
                wD256[:, 2 * i + h, :], scr_d256[:, (2 * i) * 128 + h * 128:(2 * i) * 128 + (h + 1) * 128]
            )

    # ---------------- cap prep (per quarter) -> capT tiles ---------------------
    scr_c128 = dram.tile([B, C], BF16, tag="scr_c128")
    scr_c256 = dram.tile([B, C], BF16, tag="scr_c256")
    capT128 = []  # per quarter: [128, 8, BQ]  (c-chunk j, b)
    capT256 = []
    for q in range(NQ):
        c128q = persist.tile([128, 8, BQ], BF16, tag=f"capT128_{q}")
        c256q = persist.tile([128, 8, BQ], BF16, tag=f"capT256_{q}")
        capT128.append(c128q)
        capT256.append(c256q)
        for r in range(4):  # row-tiles within quarter
            row0 = q * BQ + r * 128
            x = norm.tile([128, C], F32, tag="cap_in")
            nc.sync.dma_start(x[:], cap_d[row0:row0 + 128, :])
            n128 = norm.tile([128, C], BF16, tag="cap_n128")
            n256 = norm.tile([128, C], BF16, tag="cap_n256")
            normalize_tile(x, n128, n256)
            nc.sync.dma_start(scr_c128[row0:row0 + 128, :], n128[:])
            nc.sync.dma_start(scr_c256[row0:row0 + 128, :], n256[:])
        for j in range(8):
            nc.sync.dma_start_transpose(
                c128q[:, j, :], scr_c128[q * BQ:(q + 1) * BQ, j * 128:(j + 1) * 128]
            )
            nc.sync.dma_start_transpose(
                c256q[:, j, :], scr_c256[q * BQ:(q + 1) * BQ, j * 128:(j + 1) * 128]
            )

    # ---------------- main loop ------------------------------------------------
    for at in range(2):
        asl = slice(at * 128, (at + 1) * 128)
        for q in range(NQ):
            m_stage = stage.tile([128, 12, BQ], BF16, tag="m_stage")
            # ---- 128-blocks: t = cap chunk, v pairs (2g+i) ----
            for t in range(8):
                m_g = []
                for g in range(2):
                    pL = psum.tile([128, 2, BQ], F32, tag="pL")
                    pD = psum.tile([128, 2, BQ], F32, tag="pD")
                    for i in range(2):
                        pair = 2 * g + i
                        rhs = capT128[q][:, t, :]
                        nc.tensor.matmul(pL[:, i, :], wL128[:, pair, asl], rhs,
                                         start=True, stop=True)
                        nc.tensor.matmul(pD[:, i, :], wD128[:, pair, asl], rhs,
                                         start=True, stop=True)
                    r = drain.tile([128, 2, BQ], BF16, tag="r")
                    nc.scalar.activation(r[:], pD[:], Act.Relu)
                    m = drain.tile([128, 2, BQ], BF16, tag="m", bufs=4)
                    nc.vector.tensor_tensor(m[:], r[:], pL[:], op=Alu.add)
                    m_g.append(m)
                mm = drain.tile([128, 2, BQ], BF16, tag="mm")
                nc.vector.tensor_tensor(mm[:], m_g[0][:], m_g[1][:], op=Alu.max)
                nc.vector.tensor_tensor(
                    m_stage[:, t, :], mm[:, 0, :], mm[:, 1, :], op=Alu.max
                )
            # ---- 256-blocks: t' = cap 256-chunk, v' pairs ----
            for tp in range(4):
                pL = psum.tile([128, 2, BQ], F32, tag="pL")
                pD = psum.tile([128, 2, BQ], F32, tag="pD")
                for i in range(2):
                    for h in range(2):
                        rhs = capT256[q][:, 2 * tp + h, :]
                        nc.tensor.matmul(pL[:, i, :], wL256[:, 2 * i + h, asl], rhs,
                                         start=(h == 0), stop=(h == 1))
                        nc.tensor.matmul(pD[:, i, :], wD256[:, 2 * i + h, asl], rhs,
                                         start=(h == 0), stop=(h == 1))
                r = drain.tile([128, 2, BQ], BF16, tag="r")
                nc.scalar.activation(r[:], pD[:], Act.Relu)
                m = drain.tile([128, 2, BQ], BF16, tag="m", bufs=4)
                nc.vector.tensor_tensor(m[:], r[:], pL[:], op=Alu.add)
                nc.vector.tensor_tensor(
                    m_stage[:, 8 + tp, :], m[:, 0, :], m[:, 1, :], op=Alu.max
                )
            # ---- t-sum and writeback ----
            acc = drain.tile([128, BQ], F32, tag="acc")
            nc.vector.reduce_sum(acc[:], m_stage.transpose([0, 2, 1]), axis=Ax.X)
            nc.sync.dma_start(out_d[asl, q * BQ:(q + 1) * BQ], acc[:])


_NC_CACHE = None




# ---------------------------------------------------------------------------
# Workaround: this container's walrus build rejects instructions with more
# than one sync-wait condition ("Too many sync wait commands").  Split the
# extra waits onto sequencer-only RegisterMove carrier instructions in a BIR
# post-pass, and monkeypatch the compile entry points to apply it.
import json as _json


def _split_multiwaits(bir_bytes: bytes) -> bytes:
    m = _json.loads(bir_bytes)
    uid = [0]

    def carrier(engine, wait, debug):
        uid[0] += 1
        return {
            "debug": debug,
            "engine": engine,
            "ins": [{"dtype": "int32", "kind": "imm_value", "value": 0}],
            "outs": [{"dtype": "int32", "kind": "register_access",
                      "regref": f"{engine}_zero"}],
            "name": f"I-wsplit-{uid[0]}",
            "opcode": "RegisterMove",
            "sync_info": {"on_update": [], "on_wait": [wait]},
        }

    for f in m["functions"]:
        for bb in f["blocks"]:
            out = []
            for inst in bb["instructions"]:
                si = inst.get("sync_info")
                waits = (si or {}).get("on_wait") or []
                eng = inst.get("engine")
                if len(waits) > 1 and eng and eng != "Unassigned":
                    for w in waits[:-1]:
                        out.append(carrier(eng, w, inst.get("debug", 0)))
                    si["on_wait"] = [waits[-1]]
                out.append(inst)
            bb["instructions"] = out
    return _json.dumps(m).encode()


def _install_birpatch():
    import concourse.bass_utils as bu
    import concourse.bass2jax as b2j

    if getattr(bu.compile_bir_kernel, "_wsplit_wrapped", False):
        return
    orig = bu.compile_bir_kernel

    def wrapped(bir_json: bytes, tmpdir: str, neff_name="file.neff"):
        return orig(_split_multiwaits(bir_json), tmpdir, neff_name=neff_name)

    wrapped._wsplit_wrapped = True
    bu.compile_bir_kernel = wrapped
    b2j.compile_bir_kernel = wrapped


def kernel(img_emb: np.ndarray, cap_emb: np.ndarray) -> np.ndarray:
    _install_birpatch()
    from concourse.bass_utils import run_bass_kernel_spmd

    global _NC_CACHE
    if _NC_CACHE is None:
        _NC_CACHE = _build_kernel()
    nc = _NC_CACHE

    img = np.ascontiguousarray(np.asarray(img_emb, dtype=np.float32))
    cap = np.ascontiguousarray(np.asarray(cap_emb, dtype=np.float32))
    in_maps = [
        {"img": img[k * A_PER:(k + 1) * A_PER], "cap": cap} for k in range(N_CORES)
    ]
    res = run_bass_kernel_spmd(nc, in_maps, core_ids=list(range(N_CORES)))
    return np.concatenate([r["sims"] for r in res.results], axis=0)


if __name__ == "__main__":
    rng = np.random.default_rng(0)
    img = rng.normal(size=(A, C)).astype(np.float32)
    cap = rng.normal(size=(B, C)).astype(np.float32)
    out = kernel(img, cap)
    print("out", out.shape, out.dtype, float(out.min()), float(out.max()))
